# revision 8
# baseline (speedup 1.0000x reference)
import sys

if "/opt/trn_rl_repo" not in sys.path:
    sys.path.insert(0, "/opt/trn_rl_repo")

import numpy as np

import concourse.bacc as bacc
import concourse.bass as bass
import concourse.mybir as mybir
import concourse.tile as tile
from concourse.bass_utils import run_bass_kernel_spmd
from concourse.masks import make_identity

# Problem constants (hardcoded per contract)
B, S, H = 4, 4096, 2048
HH = H // 2  # 1024
RANKS = [4, 8, 16]
SCALING = 16.0 / max(RANKS)  # 1.0
RESIDUAL_SCALE = 1.0
THR = [0.3, 0.7]
N_CORES = 8
R = (B * S) // N_CORES  # 2048 rows per core
P = 128
NT = R // P  # 16 row tiles per core
KC = H // P  # 16 col chunks
RP = 32  # padded concat rank (4+8+16=28 -> 32)
F32 = mybir.dt.float32

_cache = {}


def _emit_body(nc, tc, rep, cst, src_k, src_v, dst_k, dst_v,
               w1, b1, w2, b2):
    """Emit one full kernel iteration reading src_*, writing dst_*."""
    sfx = f"_{rep}"
    with tc.tile_pool(name="ph1" + sfx, bufs=1) as ph1:
        partial_sb = ph1.tile([1, H], F32)
        gath_sb = ph1.tile([N_CORES, H], F32)
        xmt_sb = ph1.tile([P, KC * B], F32)
        ht_sb = ph1.tile([P, (HH // P) * B], F32)
        imp_sb = ph1.tile([1, B], F32)
        avg_sb = ph1.tile([1, 1], F32)
        s1_sb = ph1.tile([1, 1], F32)
        s2_sb = ph1.tile([1, 1], F32)
        m1_sb = ph1.tile([1, RP], F32)
        m2_sb = ph1.tile([1, RP], F32)
        mask_sb = ph1.tile([1, RP], F32)
        maskt_sb = ph1.tile([RP, 1], F32)
        bmk_sb = ph1.tile([RP, H], F32)
        bmv_sb = ph1.tile([RP, H], F32)

        # ---- Phase 1: column-sum of this core's keys slab ----
        with tc.tile_pool(name="p1x" + sfx, bufs=3) as p1x, \
             tc.tile_pool(name="p1ps" + sfx, bufs=4, space="PSUM") as p1ps:
            cs = [p1ps.tile([1, 512], F32, tag="cs", name=f"cs{n}{sfx}")
                  for n in range(4)]
            for t in range(NT):
                xt = p1x.tile([P, H], F32, tag="x")
                nc.sync.dma_start(out=xt[:], in_=src_k[t * P:(t + 1) * P, :])
                for n in range(4):
                    nc.tensor.matmul(cs[n][:], cst["ones128"][:],
                                     xt[:, n * 512:(n + 1) * 512],
                                     start=(t == 0), stop=(t == NT - 1),
                                     skip_group_check=True)
            for n in range(4):
                nc.scalar.copy(partial_sb[:, n * 512:(n + 1) * 512],
                               cs[n][:])

        # ---- AllGather partials across the 8 cores ----
        with tc.tile_pool(name="dram" + sfx, bufs=1, space="DRAM") as dram:
            cc_in = dram.tile([1, H], F32)
            cc_out = dram.tile([N_CORES, H], F32)
            nc.gpsimd.dma_start(out=cc_in[:], in_=partial_sb[:])
            nc.gpsimd.collective_compute(
                "AllGather", mybir.AluOpType.bypass,
                replica_groups=[list(range(N_CORES))],
                ins=[cc_in.opt()], outs=[cc_out.opt()])
            nc.gpsimd.dma_start(out=gath_sb[:], in_=cc_out[:])

        # ---- x_meanT [H, B] = gathered^T @ fsel ----
        with tc.tile_pool(name="p1b" + sfx, bufs=2, space="PSUM") as p1b:
            for k in range(KC):
                ps = p1b.tile([P, B], F32, tag="xm")
                nc.tensor.matmul(ps[:], gath_sb[:, k * P:(k + 1) * P],
                                 cst["fsel_sb"][:], start=True, stop=True)
                nc.scalar.copy(xmt_sb[:, k * B:(k + 1) * B], ps[:])

            # ---- MLP layer 1: hT = relu(w1^T @ x_mean + b1) ----
            with tc.tile_pool(name="w1p" + sfx, bufs=1) as w1p:
                w1_sb = w1p.tile([P, KC * HH], F32)
                for k in range(KC):
                    nc.sync.dma_start(out=w1_sb[:, k * HH:(k + 1) * HH],
                                      in_=w1[k * P:(k + 1) * P, :])
                for m in range(HH // P):
                    ps = p1b.tile([P, B], F32, tag="h")
                    for k in range(KC):
                        nc.tensor.matmul(
                            ps[:],
                            w1_sb[:, k * HH + m * P:k * HH + (m + 1) * P],
                            xmt_sb[:, k * B:(k + 1) * B],
                            start=(k == 0), stop=(k == KC - 1))
                    nc.scalar.activation(
                        ht_sb[:, m * B:(m + 1) * B], ps[:],
                        mybir.ActivationFunctionType.Relu,
                        bias=cst["b1_sb"][:, m:m + 1])

            # ---- MLP layer 2 + sigmoid ----
            ps_i = p1b.tile([1, B], F32, tag="imp")
            for c in range(HH // P):
                nc.tensor.matmul(ps_i[:], cst["w2_sb"][:, c:c + 1],
                                 ht_sb[:, c * B:(c + 1) * B],
                                 start=(c == 0), stop=(c == HH // P - 1))
            nc.scalar.activation(imp_sb[:], ps_i[:],
                                 mybir.ActivationFunctionType.Sigmoid,
                                 bias=cst["b2_sb"][0:1, 0:1])

            # ---- avg, thresholds, rank-block mask ----
            nc.vector.tensor_reduce(avg_sb[:], imp_sb[:],
                                    mybir.AxisListType.X,
                                    mybir.AluOpType.add)
            nc.scalar.mul(avg_sb[:], avg_sb[:], 1.0 / B)
            nc.vector.tensor_scalar(s1_sb[:], avg_sb[:], THR[0], None,
                                    op0=mybir.AluOpType.is_ge)
            nc.vector.tensor_scalar(s2_sb[:], avg_sb[:], THR[1], None,
                                    op0=mybir.AluOpType.is_ge)
            maskc_sb = cst["maskc_sb"]
            nc.vector.tensor_scalar(m1_sb[:], maskc_sb[:, RP:2 * RP],
                                    s1_sb[0:1, 0:1], None,
                                    op0=mybir.AluOpType.mult)
            nc.vector.tensor_scalar(m2_sb[:], maskc_sb[:, 2 * RP:3 * RP],
                                    s2_sb[0:1, 0:1], None,
                                    op0=mybir.AluOpType.mult)
            nc.vector.tensor_tensor(mask_sb[:], maskc_sb[:, 0:RP],
                                    m1_sb[:], op=mybir.AluOpType.add)
            nc.vector.tensor_tensor(mask_sb[:], mask_sb[:], m2_sb[:],
                                    op=mybir.AluOpType.add)
            ps_mt = p1b.tile([RP, 1], F32, tag="mt")
            nc.tensor.matmul(ps_mt[:], mask_sb[:], cst["one1"][:],
                             start=True, stop=True)
            nc.scalar.copy(maskt_sb[:], ps_mt[:])
            nc.vector.tensor_scalar(bmk_sb[:], cst["bk_sb"][:],
                                    maskt_sb[:, 0:1], None,
                                    op0=mybir.AluOpType.mult)
            nc.vector.tensor_scalar(bmv_sb[:], cst["bv_sb"][:],
                                    maskt_sb[:, 0:1], None,
                                    op0=mybir.AluOpType.mult)

        # ---- Phase 2: out = x + (x @ Acat) @ Bmask for both tensors ----
        with tc.tile_pool(name="xp" + sfx, bufs=3) as xp, \
             tc.tile_pool(name="xtp" + sfx, bufs=2) as xtp, \
             tc.tile_pool(name="ttp" + sfx, bufs=2) as ttp, \
             tc.tile_pool(name="op" + sfx, bufs=3) as op, \
             tc.tile_pool(name="trp" + sfx, bufs=3, space="PSUM") as trp, \
             tc.tile_pool(name="tp" + sfx, bufs=2, space="PSUM") as tp, \
             tc.tile_pool(name="pso" + sfx, bufs=2, space="PSUM") as pso:
            for x_dram, a_sb, bm_sb, o_dram in (
                    (src_k, cst["ak_sb"], bmk_sb, dst_k),
                    (src_v, cst["av_sb"], bmv_sb, dst_v)):
                for t in range(NT):
                    xt = xp.tile([P, H], F32, tag="x")
                    nc.sync.dma_start(out=xt[:],
                                      in_=x_dram[t * P:(t + 1) * P, :])
                    xtt = xtp.tile([P, H], F32, tag="xt")
                    for k in range(KC):
                        ps_tr = trp.tile([P, P], F32, tag="tr")
                        nc.tensor.transpose(ps_tr[:],
                                            xt[:, k * P:(k + 1) * P],
                                            cst["id128"][:])
                        if k % 2 == 0:
                            nc.vector.tensor_copy(
                                xtt[:, k * P:(k + 1) * P], ps_tr[:])
                        else:
                            nc.scalar.copy(
                                xtt[:, k * P:(k + 1) * P], ps_tr[:])
                    ps_t = tp.tile([RP, P], F32, tag="t")
                    for k in range(KC):
                        nc.tensor.matmul(ps_t[:],
                                         a_sb[:, k * RP:(k + 1) * RP],
                                         xtt[:, k * P:(k + 1) * P],
                                         start=(k == 0),
                                         stop=(k == KC - 1))
                    tt = ttp.tile([RP, P], F32, tag="tt")
                    nc.vector.tensor_copy(tt[:], ps_t[:])
                    ot = op.tile([P, H], F32, tag="o")
                    for n in range(4):
                        ps_o = pso.tile([P, 512], F32, tag="o")
                        nc.tensor.matmul(ps_o[:], tt[:],
                                         bm_sb[:, n * 512:(n + 1) * 512],
                                         start=True, stop=True)
                        nc.vector.tensor_tensor(
                            ot[:, n * 512:(n + 1) * 512], ps_o[:],
                            xt[:, n * 512:(n + 1) * 512],
                            op=mybir.AluOpType.add)
                    nc.sync.dma_start(out=o_dram[t * P:(t + 1) * P, :],
                                      in_=ot[:])


def _build_program(repeat=1):
    nc = bacc.Bacc("TRN2", target_bir_lowering=False, debug=False,
                   num_devices=N_CORES)

    k_slab = nc.dram_tensor("k_slab", [R, H], F32, kind="ExternalInput").ap()
    v_slab = nc.dram_tensor("v_slab", [R, H], F32, kind="ExternalInput").ap()
    w1 = nc.dram_tensor("w1", [H, HH], F32, kind="ExternalInput").ap()
    b1 = nc.dram_tensor("b1", [HH, 1], F32, kind="ExternalInput").ap()
    w2 = nc.dram_tensor("w2", [HH, 1], F32, kind="ExternalInput").ap()
    b2 = nc.dram_tensor("b2", [1, 1], F32, kind="ExternalInput").ap()
    acat_k = nc.dram_tensor("acat_k", [H, RP], F32, kind="ExternalInput").ap()
    acat_v = nc.dram_tensor("acat_v", [H, RP], F32, kind="ExternalInput").ap()
    bcat_k = nc.dram_tensor("bcat_k", [RP, H], F32, kind="ExternalInput").ap()
    bcat_v = nc.dram_tensor("bcat_v", [RP, H], F32, kind="ExternalInput").ap()
    fsel = nc.dram_tensor("fsel", [N_CORES, B], F32, kind="ExternalInput").ap()
    maskc = nc.dram_tensor("maskc", [1, 3 * RP], F32,
                           kind="ExternalInput").ap()
    ck_slab = nc.dram_tensor("ck_slab", [R, H], F32, kind="ExternalOutput").ap()
    cv_slab = nc.dram_tensor("cv_slab", [R, H], F32, kind="ExternalOutput").ap()

    with tile.TileContext(nc) as tc:
        with tc.tile_pool(name="const", bufs=1) as const:
            cst = {}
            ones128 = const.tile([P, 1], F32)
            nc.vector.memset(ones128[:], 1.0)
            one1 = const.tile([1, 1], F32)
            nc.vector.memset(one1[:], 1.0)
            id128 = const.tile([P, P], F32)
            make_identity(nc, id128[:])
            fsel_sb = const.tile([N_CORES, B], F32)
            nc.sync.dma_start(out=fsel_sb[:], in_=fsel[:])
            maskc_sb = const.tile([1, 3 * RP], F32)
            nc.sync.dma_start(out=maskc_sb[:], in_=maskc[:])
            b2_sb = const.tile([1, 1], F32)
            nc.sync.dma_start(out=b2_sb[:], in_=b2[:])
            w2_sb = const.tile([P, HH // P], F32)
            b1_sb = const.tile([P, HH // P], F32)
            for c in range(HH // P):
                nc.sync.dma_start(out=w2_sb[:, c:c + 1],
                                  in_=w2[c * P:(c + 1) * P, :])
                nc.sync.dma_start(out=b1_sb[:, c:c + 1],
                                  in_=b1[c * P:(c + 1) * P, :])
            ak_sb = const.tile([P, KC * RP], F32)
            av_sb = const.tile([P, KC * RP], F32)
            for k in range(KC):
                nc.sync.dma_start(out=ak_sb[:, k * RP:(k + 1) * RP],
                                  in_=acat_k[k * P:(k + 1) * P, :])
                nc.sync.dma_start(out=av_sb[:, k * RP:(k + 1) * RP],
                                  in_=acat_v[k * P:(k + 1) * P, :])
            bk_sb = const.tile([RP, H], F32)
            bv_sb = const.tile([RP, H], F32)
            nc.sync.dma_start(out=bk_sb[:], in_=bcat_k[:])
            nc.sync.dma_start(out=bv_sb[:], in_=bcat_v[:])
            cst = dict(ones128=ones128, one1=one1, id128=id128,
                       fsel_sb=fsel_sb, maskc_sb=maskc_sb, b2_sb=b2_sb,
                       w2_sb=w2_sb, b1_sb=b1_sb, ak_sb=ak_sb, av_sb=av_sb,
                       bk_sb=bk_sb, bv_sb=bv_sb)

            if repeat == 1:
                _emit_body(nc, tc, 0, cst, k_slab, v_slab, ck_slab, cv_slab,
                           w1, b1, w2, b2)
            else:
                with tc.tile_pool(name="ppd", bufs=1, space="DRAM") as ppd:
                    ppk = [ppd.tile([R, H], F32, name=f"ppk{i}")
                           for i in range(2)]
                    ppv = [ppd.tile([R, H], F32, name=f"ppv{i}")
                           for i in range(2)]
                    for rep in range(repeat):
                        src_k = k_slab if rep == 0 else ppk[rep % 2][:]
                        src_v = v_slab if rep == 0 else ppv[rep % 2][:]
                        last = rep == repeat - 1
                        dst_k = ck_slab if last else ppk[(rep + 1) % 2][:]
                        dst_v = cv_slab if last else ppv[(rep + 1) % 2][:]
                        _emit_body(nc, tc, rep, cst, src_k, src_v,
                                   dst_k, dst_v, w1, b1, w2, b2)

    nc.compile()
    return nc


def _get_program():
    if "nc" not in _cache:
        _cache["nc"] = _build_program()
    return _cache["nc"]


def _prep_in_maps(inputs):
    f32 = np.float32
    keys = np.asarray(inputs["keys"], dtype=f32)
    values = np.asarray(inputs["values"], dtype=f32)
    kf = np.ascontiguousarray(keys.reshape(B * S, H))
    vf = np.ascontiguousarray(values.reshape(B * S, H))
    scale = SCALING * RESIDUAL_SCALE

    def cat_a(a0, a1, a2):
        out = np.zeros((H, RP), dtype=f32)
        out[:, 0:4] = a0
        out[:, 4:12] = a1
        out[:, 12:28] = a2
        return out

    def cat_b(b0, b1_, b2_):
        out = np.zeros((RP, H), dtype=f32)
        out[0:4, :] = b0
        out[4:12, :] = b1_
        out[12:28, :] = b2_
        return out * scale

    acat_k = cat_a(inputs["kA0"], inputs["kA1"], inputs["kA2"])
    acat_v = cat_a(inputs["vA0"], inputs["vA1"], inputs["vA2"])
    bcat_k = cat_b(inputs["kB0"], inputs["kB1"], inputs["kB2"])
    bcat_v = cat_b(inputs["vB0"], inputs["vB1"], inputs["vB2"])

    fsel = np.zeros((N_CORES, B), dtype=f32)
    for c in range(N_CORES):
        fsel[c, c // 2] = 1.0 / S

    u = np.zeros((3, RP), dtype=f32)
    u[0, 0:4] = 1.0
    u[1, 4:12] = 1.0
    u[2, 12:28] = 1.0
    maskc = np.concatenate([u[0], u[1] - u[0], u[2] - u[1]]).astype(f32)

    common = {
        "w1": np.ascontiguousarray(inputs["w1"], dtype=f32),
        "b1": np.ascontiguousarray(
            np.asarray(inputs["b1"], dtype=f32).reshape(HH, 1)),
        "w2": np.ascontiguousarray(inputs["w2"], dtype=f32),
        "b2": np.ascontiguousarray(
            np.asarray(inputs["b2"], dtype=f32).reshape(1, 1)),
        "acat_k": acat_k, "acat_v": acat_v,
        "bcat_k": bcat_k, "bcat_v": bcat_v,
        "fsel": fsel, "maskc": maskc.reshape(1, 3 * RP),
    }
    return [
        dict(common,
             k_slab=np.ascontiguousarray(kf[c * R:(c + 1) * R]),
             v_slab=np.ascontiguousarray(vf[c * R:(c + 1) * R]))
        for c in range(N_CORES)
    ]


def kernel(**inputs):
    in_maps = _prep_in_maps(inputs)
    nc = _get_program()
    res = run_bass_kernel_spmd(nc, in_maps, list(range(N_CORES)),
                               **_cache.get("run_kwargs", {}))
    _cache["last_result"] = res
    ck = np.concatenate([res.results[c]["ck_slab"] for c in range(N_CORES)],
                        axis=0).reshape(B, S, H)
    cv = np.concatenate([res.results[c]["cv_slab"] for c in range(N_CORES)],
                        axis=0).reshape(B, S, H)
    return ck, cv
